# revision 1
# baseline (speedup 1.0000x reference)
"""Trainium2 Bass kernel for nn_DecoderBlock (self-attn + cross-attn + FFN, post-LN).

Sharding: 8 cores = (batch b in {0,1}) x (query block qi in {0..3} of 512 rows).
Each core computes its 512 output rows end-to-end; K/V work over the full
sequence is replicated inside a batch (no cross-core communication).

All on-chip activations are kept transposed [d, s] so every GEMM consumes
natural weight layouts and no on-device transposes are needed. The host
transposes x/enc on the way in and the output on the way out.

Attention uses the transposed layout: S^T[k,q] = K_h^T(dk,k)^T-free matmuls
with two heads packed into the 128-row PE array via tile_position; softmax is
exp(s/8 - 4) with the normalizer produced by an extra ones-column on V
(M=65 matmul) and divided out after accumulation. Causal masking is applied
as a per-core 0/1 mask multiply on the exp tiles (mask content differs per
core; the program is identical across cores). The cross-attention key mask
(src_mask) is folded into the V rows instead.

All matmuls run in float32r (measured ~1.5e-4 rel err vs fp32).
"""

import numpy as np

import concourse.bass as bass
import concourse.mybir as mybir
import concourse.tile as tile
from concourse import bacc
from concourse.bass import ds
from concourse.bass_utils import run_bass_kernel_spmd

F32 = mybir.dt.float32
F32R = mybir.dt.float32r
AF = mybir.ActivationFunctionType
ALU = mybir.AluOpType

B, S, D, H, DK, DFF = 2, 2048, 1024, 16, 64, 4096
NCORES = 8
QS = 512            # query rows per core
DC = D // 128       # 8 d-chunks
FC = DFF // 128     # 32 dff-chunks
PANEL = 512         # kpos panel size
NPANEL = S // PANEL # 4
NSC = PANEL // 128  # 4 kpos chunks per panel
NHP = H // 2        # 8 head pairs
LN_EPS = 1e-5
EXP_BIAS = -4.0     # exp(s/8 - 4): overflow safety; cancels in the normalizer


def _dchunks(ap):
    """[D-like, N] dram AP -> [128, chunks, N] (partition = row % 128)."""
    return ap.rearrange("(c p) n -> p c n", p=128)


def _wpairs(ap):
    """[K, M] weight AP -> [128, K//128, M]; slice pairs of K-chunks."""
    return ap.rearrange("(c p) m -> p c m", p=128)


tap_layout = {}


def _build(tap=None):
    nc = bacc.Bacc("TRN2", target_bir_lowering=False, debug=False,
                   num_devices=NCORES)

    def inp(name, shape):
        return nc.dram_tensor(name, shape, F32, kind="ExternalInput").ap()

    xoT = inp("xoT", [D, QS])          # x[b].T[:, q0:q0+QS]
    xT = inp("xT", [D, S])             # x[b].T
    eT = inp("eT", [D, S])             # enc[b].T
    msk = inp("msk", [S // 128, 128, QS])  # per-core causal mask (k-chunk, k, q)
    vms = inp("vms", [128, S // 128])  # src_mask per kpos, per-partition layout
    w_sa = {t: inp(f"w_sa{t}", [D, D]) for t in "qkvo"}
    w_ca = {t: inp(f"w_ca{t}", [D, D]) for t in "qkvo"}
    w_ff1 = inp("w_ff1", [D, DFF])
    w_ff2 = inp("w_ff2", [DFF, D])
    fb1 = inp("fb1", [128, FC])        # ff_b1 in [128, chunk] layout
    fb2 = inp("fb2", [128, DC])
    lnb = inp("lnb", [128, 6 * DC])    # g1,b1,g2,b2,g3,b3 packed
    outT = nc.dram_tensor("outT", [128, DC, QS], F32, kind="ExternalOutput").ap()
    dbg = nc.dram_tensor("dbg", [128, 40, QS], F32, kind="ExternalOutput").ap() \
        if tap else None
    tapped = []

    def tapit(name, ap):
        if tap and (tap == "all" or name in tap):
            tapped.append((name, ap))

    with tile.TileContext(nc) as tc:
        with tc.tile_pool(name="glob", bufs=1) as G, \
             tc.tile_pool(name="acts", bufs=2) as ACTS, \
             tc.tile_pool(name="ps2", bufs=2, space="PSUM") as PS2, \
             tc.tile_pool(name="ps1", bufs=1, space="PSUM") as PS1:

            ones_f = G.tile([128, 64], F32)
            nc.vector.memset(ones_f[:], 1.0)
            ones = G.tile([128, 1], F32R)
            nc.vector.tensor_copy(ones[:], ones_f[:, 0:1])
            cexpb = G.tile([128, 1], F32)
            nc.vector.memset(cexpb[:], EXP_BIAS)
            cleps = G.tile([128, 1], F32)
            nc.vector.memset(cleps[:], LN_EPS)
            lnbt = G.tile([128, 6 * DC], F32)
            nc.sync.dma_start(lnbt[:], lnb)
            fb1t = G.tile([128, FC], F32)
            nc.sync.dma_start(fb1t[:], fb1)
            fb2t = G.tile([128, DC], F32)
            nc.sync.dma_start(fb2t[:], fb2)
            vmst = G.tile([128, S // 128], F32)
            nc.sync.dma_start(vmst[:], vms)
            stats = G.tile([1, 8, QS], F32)

            def proj_from_dram(wdram, rhs, evict, n_mc=DC, n_kc=DC, wtag="w"):
                """psum[mc] = sum_kc w[kc,mc-chunk].T @ rhs[:,kc,:]; evict(mc, psum)."""
                wre = _wpairs(wdram)
                for mc in range(n_mc):
                    ps = PS2.tile([128, QS], F32, tag="pj")
                    for k2 in range(n_kc // 2):
                        wt = WPOOL.tile([128, 2, 128], F32R, tag=wtag)
                        nc.sync.dma_start(
                            wt[:],
                            wre[:, 2 * k2:2 * k2 + 2, ds(mc * 128, 128)].bitcast(F32R))
                        for j in range(2):
                            kc = 2 * k2 + j
                            nc.tensor.matmul(ps[:], wt[:, j, :], rhs[:, kc, :],
                                             start=(kc == 0), stop=(kc == n_kc - 1))
                    evict(mc, ps)

            def layernorm(xpre, gcol, bcol, out, TMP):
                """out[:,mc,:] = (xpre - mu)/sd * g + b, stats over d (partition+chunks)."""
                pmu = PS2.tile([1, QS], F32, tag="pj")
                for kc in range(DC):
                    nc.tensor.matmul(pmu[:], ones[:], xpre[:, kc, :],
                                     start=(kc == 0), stop=(kc == DC - 1))
                pm2 = PS2.tile([1, QS], F32, tag="pj")
                for kc in range(DC):
                    sq = TMP.tile([128, QS], F32R, tag="sq")
                    nc.scalar.activation(sq[:], xpre[:, kc, :], AF.Square)
                    nc.tensor.matmul(pm2[:], ones[:], sq[:],
                                     start=(kc == 0), stop=(kc == DC - 1))
                mu = stats[0:1, 0, :]
                ex2 = stats[0:1, 1, :]
                var = stats[0:1, 2, :]
                sd = stats[0:1, 3, :]
                rstd = stats[0:1, 4, :]
                nc.vector.tensor_scalar_mul(mu, pmu[:], 1.0 / D)
                nc.vector.tensor_scalar_mul(ex2, pm2[:], 1.0 / D)
                nc.vector.tensor_tensor(var, mu, mu, ALU.mult)
                nc.vector.tensor_sub(var, ex2, var)
                nc.scalar.activation(sd, var, AF.Sqrt, bias=cleps[0:1, :])
                nc.vector.reciprocal(rstd, sd)
                mub = TMP.tile([128, QS], F32, tag="mub")
                nc.gpsimd.partition_broadcast(mub[:], mu)
                rsb = TMP.tile([128, QS], F32, tag="rsb")
                nc.gpsimd.partition_broadcast(rsb[:], rstd)
                for mc in range(DC):
                    t = TMP.tile([128, QS], F32, tag="t")
                    nc.vector.tensor_sub(t[:], xpre[:, mc, :], mub[:])
                    nc.vector.tensor_mul(t[:], t[:], rsb[:])
                    nc.vector.tensor_scalar(
                        out=out[:, mc, :], in0=t[:],
                        scalar1=gcol[:, mc:mc + 1], scalar2=bcol[:, mc:mc + 1],
                        op0=ALU.mult, op1=ALU.add)

            def attention(qsrc, kv_dram, w, res, gcol, bcol, masked):
                nonlocal WPOOL
                sfx = "sa" if masked else "ca"
                kvre = _dchunks(kv_dram)
                with tc.tile_pool(name="attn", bufs=1) as A:
                    QT = A.tile([128, DC, QS], F32R)
                    oacc = A.tile([65, H, QS], F32)
                    with tc.tile_pool(name="wq", bufs=4) as WPOOL:
                        def evq(mc, ps):
                            nc.vector.tensor_copy(QT[:, mc, :], ps[:])
                        proj_from_dram(w["q"], qsrc, evq)
                    tapit("QT" + ("sa" if masked else "ca"), QT)

                    with tc.tile_pool(name="panel", bufs=1) as P, \
                         tc.tile_pool(name="wkp", bufs=4) as WPOOL, \
                         tc.tile_pool(name="wvp", bufs=3) as WV, \
                         tc.tile_pool(name="pp", bufs=2) as PP, \
                         tc.tile_pool(name="xpp", bufs=2) as XPP:
                        for p in range(NPANEL):
                            xp = XPP.tile([128, DC, PANEL], F32R, tag="xp")
                            nc.sync.dma_start(
                                xp[:], kvre[:, :, ds(p * PANEL, PANEL)].bitcast(F32R))
                            KT = P.tile([128, DC, PANEL], F32R, tag="kt")

                            def evk(mc, ps):
                                nc.vector.tensor_copy(KT[:, mc, :], ps[:])
                            proj_from_dram(w["k"], xp, evk)
                            if p == 0:
                                tapit("KT" + ("sa" if masked else "ca"), KT)

                            v1 = P.tile([128, NSC, H, DK + 1], F32R, tag="v1")
                            nc.vector.tensor_copy(
                                v1[:, :, :, DK],
                                ones_f[:].rearrange("p (a b) -> p a b", a=NSC))
                            wvre = _dchunks(w["v"])
                            for nh in range(2):
                                pss = [PS2.tile([128, QS], F32, tag="s0", name="vps0"),
                                       PS2.tile([128, QS], F32, tag="s1", name="vps1"),
                                       PS1.tile([128, QS], F32, tag="o0", name="vps2"),
                                       PS1.tile([128, QS], F32, tag="o1", name="vps3")]
                                for kc in range(DC):
                                    wvt = WV.tile([128, PANEL], F32R, tag="wv")
                                    nc.sync.dma_start(
                                        wvt[:],
                                        wvre[:, kc, ds(nh * 512, 512)].bitcast(F32R))
                                    for sc in range(NSC):
                                        nc.tensor.matmul(
                                            pss[sc][:], xp[:, kc, ds(sc * 128, 128)],
                                            wvt[:], start=(kc == 0), stop=(kc == DC - 1))
                                for sc in range(NSC):
                                    nc.vector.tensor_copy(
                                        v1[:, sc, nh * 8:(nh + 1) * 8, 0:DK],
                                        pss[sc][:].rearrange("p (a b) -> p a b", a=8))
                            if not masked:
                                # fold src_mask into V rows (incl. ones column)
                                for sc in range(NSC):
                                    nc.vector.tensor_scalar_mul(
                                        v1[:, sc, :, :], v1[:, sc, :, :],
                                        vmst[:, p * NSC + sc:p * NSC + sc + 1])
                            if masked:
                                mt = P.tile([128, NSC, QS], F32, tag="mk")
                                nc.sync.dma_start(
                                    mt[:],
                                    msk[ds(p * NSC, NSC)].rearrange("c p q -> p c q"))
                            for hp in range(NHP):
                                po0 = PS1.tile([65, QS], F32, tag="o0")
                                po1 = PS1.tile([65, QS], F32, tag="o1")
                                for sc in range(NSC):
                                    ps0 = PS2.tile([128, QS], F32, tag="s0")
                                    ps1 = PS2.tile([128, QS], F32, tag="s1")
                                    nc.tensor.matmul(
                                        ps0[:], KT[0:64, hp, ds(sc * 128, 128)],
                                        QT[0:64, hp, :], start=True, stop=True)
                                    nc.tensor.matmul(
                                        ps1[:], KT[64:128, hp, ds(sc * 128, 128)],
                                        QT[64:128, hp, :], start=True, stop=True,
                                        tile_position=(64, 0))
                                    p0 = PP.tile([128, QS], F32R, tag="p0")
                                    p1 = PP.tile([128, QS], F32R, tag="p1")
                                    nc.scalar.activation(p0[:], ps0[:], AF.Exp,
                                                         scale=0.125, bias=cexpb[:])
                                    nc.scalar.activation(p1[:], ps1[:], AF.Exp,
                                                         scale=0.125, bias=cexpb[:])
                                    if masked:
                                        nc.vector.tensor_mul(p0[:], p0[:], mt[:, sc, :])
                                        nc.vector.tensor_mul(p1[:], p1[:], mt[:, sc, :])
                                    nc.tensor.matmul(po0[:], v1[:, sc, 2 * hp, :],
                                                     p0[:], start=(sc == 0),
                                                     stop=(sc == NSC - 1))
                                    nc.tensor.matmul(po1[:], v1[:, sc, 2 * hp + 1, :],
                                                     p1[:], start=(sc == 0),
                                                     stop=(sc == NSC - 1))
                                if p == 0:
                                    nc.vector.tensor_copy(oacc[:, 2 * hp, :], po0[:])
                                    nc.vector.tensor_copy(oacc[:, 2 * hp + 1, :], po1[:])
                                else:
                                    nc.vector.tensor_add(oacc[:, 2 * hp, :],
                                                         oacc[:, 2 * hp, :], po0[:])
                                    nc.vector.tensor_add(oacc[:, 2 * hp + 1, :],
                                                         oacc[:, 2 * hp + 1, :], po1[:])

                    with tc.tile_pool(name="aepi", bufs=1) as E, \
                         tc.tile_pool(name="rnbp", bufs=2) as RNB, \
                         tc.tile_pool(name="tmp", bufs=2) as TMP, \
                         tc.tile_pool(name="wo", bufs=4) as WPOOL:
                        tapit("oacc" + sfx, oacc)
                        rn = E.tile([1, H, QS], F32)
                        nc.vector.reciprocal(rn[:], oacc[64:65, :, :])
                        ON = E.tile([128, DC, QS], F32R)
                        for m in range(DC):
                            rnb = RNB.tile([64, 2, QS], F32, tag="rnb")
                            nc.gpsimd.partition_broadcast(rnb[:, 0, :],
                                                          rn[0:1, 2 * m, :])
                            nc.gpsimd.partition_broadcast(rnb[:, 1, :],
                                                          rn[0:1, 2 * m + 1, :])
                            nc.vector.tensor_mul(ON[0:64, m, :],
                                                 oacc[0:64, 2 * m, :], rnb[:, 0, :])
                            nc.vector.tensor_mul(ON[64:128, m, :],
                                                 oacc[0:64, 2 * m + 1, :], rnb[:, 1, :])
                        xpre = E.tile([128, DC, QS], F32R)

                        def evo(mc, ps):
                            nc.vector.tensor_add(xpre[:, mc, :], ps[:], res[:, mc, :])
                        proj_from_dram(w["o"], ON, evo)
                        tapit("ON" + sfx, ON)
                        tapit("xpre" + sfx, xpre)
                        xnext = ACTS.tile([128, DC, QS], F32R, tag="act")
                        layernorm(xpre, gcol, bcol, xnext, TMP)
                        tapit("xn" + sfx, xnext)
                return xnext

            # ---- load own-query activations ----
            xo = ACTS.tile([128, DC, QS], F32R, tag="act")
            nc.sync.dma_start(xo[:], _dchunks(xoT).bitcast(F32R))

            WPOOL = None
            g1, b1 = lnbt[:, 0:DC], lnbt[:, DC:2 * DC]
            g2, b2 = lnbt[:, 2 * DC:3 * DC], lnbt[:, 3 * DC:4 * DC]
            g3, b3 = lnbt[:, 4 * DC:5 * DC], lnbt[:, 5 * DC:6 * DC]

            x1 = attention(xo, xT, w_sa, xo, g1, b1, masked=True)
            x2 = attention(x1, eT, w_ca, x1, g2, b2, masked=False)

            # ---- FFN ----
            with tc.tile_pool(name="ffn", bufs=1) as F, \
                 tc.tile_pool(name="tmp2", bufs=2) as TMP, \
                 tc.tile_pool(name="wf", bufs=4) as WPOOL:
                h1 = F.tile([128, FC, QS], F32R)

                def ev1(fc, ps):
                    nc.scalar.activation(h1[:, fc, :], ps[:], AF.Relu,
                                         bias=fb1t[:, fc:fc + 1])
                proj_from_dram(w_ff1, x2, ev1, n_mc=FC, n_kc=DC)

                tapit("h1a", h1[:, 0:8, :])
                tapit("h1b", h1[:, 8:16, :])
                xpre = F.tile([128, DC, QS], F32R)

                def ev2(mc, ps):
                    nc.vector.scalar_tensor_tensor(
                        out=xpre[:, mc, :], in0=ps[:],
                        scalar=fb2t[:, mc:mc + 1], in1=x2[:, mc, :],
                        op0=ALU.add, op1=ALU.add)
                proj_from_dram(w_ff2, h1, ev2, n_mc=DC, n_kc=FC)

                tapit("xpreff", xpre)
                out = F.tile([128, DC, QS], F32)
                layernorm(xpre, g3, b3, out, TMP)
                tapit("outf", out)
                tc.strict_bb_all_engine_barrier()
                for mc in range(DC):
                    nc.sync.dma_start(outT[:, mc, :], out[:, mc, :])
            if tap:
                base = 0
                tap_layout.clear()
                for name, t in tapped:
                    sh = t.shape
                    nparts = sh[0]
                    assert len(sh) == 3 and sh[2] == QS
                    tap_layout[name] = (base, sh[1], nparts)
                    for cci in range(sh[1]):
                        nc.sync.dma_start(
                            dbg[0:nparts, base + cci, :].bitcast(t.dtype),
                            t[:, cci, :])
                    base += sh[1]
                assert base <= 40

    nc.compile()
    return nc


_NC_CACHE = None


def _get_nc():
    global _NC_CACHE
    if _NC_CACHE is None:
        _NC_CACHE = _build()
    return _NC_CACHE


def _prep_in_maps(x, enc, tgt_mask, src_mask,
                  sa_wq, sa_wk, sa_wv, sa_wo,
                  ca_wq, ca_wk, ca_wv, ca_wo,
                  ff_w1, ff_b1, ff_w2, ff_b2,
                  ln1_g, ln1_b, ln2_g, ln2_b, ln3_g, ln3_b):
    f32 = np.float32

    def c(a):
        return np.ascontiguousarray(np.asarray(a), dtype=f32)

    xTb = [c(np.asarray(x)[b].T) for b in range(B)]          # [1024, 2048]
    eTb = [c(np.asarray(enc)[b].T) for b in range(B)]
    tm = np.asarray(tgt_mask)[0, 0].astype(f32).T            # [k, q]
    sm = np.asarray(src_mask)[0, 0, 0].astype(f32)           # [k]
    vms = c(sm.reshape(S // 128, 128).T)                     # [128, 16]

    def percol(v, nchunks):
        return c(np.asarray(v).reshape(nchunks, 128).T)

    lnb = c(np.concatenate(
        [percol(v, DC) for v in [ln1_g, ln1_b, ln2_g, ln2_b, ln3_g, ln3_b]],
        axis=1))
    fb1 = percol(ff_b1, FC)
    fb2 = percol(ff_b2, DC)
    shared = {
        "vms": vms, "lnb": lnb, "fb1": fb1, "fb2": fb2,
        "w_saq": c(sa_wq), "w_sak": c(sa_wk), "w_sav": c(sa_wv), "w_sao": c(sa_wo),
        "w_caq": c(ca_wq), "w_cak": c(ca_wk), "w_cav": c(ca_wv), "w_cao": c(ca_wo),
        "w_ff1": c(ff_w1), "w_ff2": c(ff_w2),
    }
    in_maps = []
    for core in range(NCORES):
        b, qi = core // 4, core % 4
        q0 = qi * QS
        m = dict(shared)
        m["xT"] = xTb[b]
        m["eT"] = eTb[b]
        m["xoT"] = c(xTb[b][:, q0:q0 + QS])
        m["msk"] = c(tm[:, q0:q0 + QS].reshape(S // 128, 128, QS))
        in_maps.append(m)
    return in_maps


def _gather_out(res):
    out = np.empty((B, S, D), dtype=np.float32)
    for core in range(NCORES):
        b, qi = core // 4, core % 4
        q0 = qi * QS
        arr = res.results[core]["outT"]  # [128, DC, QS]
        out[b, q0:q0 + QS, :] = arr.transpose(1, 0, 2).reshape(D, QS).T
    return out


def kernel(**inputs):
    in_maps = _prep_in_maps(**inputs)
    nc = _get_nc()
    res = run_bass_kernel_spmd(nc, in_maps, core_ids=list(range(NCORES)))
    return _gather_out(res)


def _profiled_run(inputs):
    """Test-only: run with NTFF tracing to get HW exec time."""
    in_maps = _prep_in_maps(**inputs)
    nc = _get_nc()
    return run_bass_kernel_spmd(nc, in_maps, core_ids=list(range(NCORES)),
                                trace=True)



# revision 7
# speedup vs baseline: 1.2341x; 1.2341x over previous
"""Trainium2 Bass kernel for nn_DecoderBlock (self-attn + cross-attn + FFN, post-LN).

Sharding: 8 cores = (batch b in {0,1}) x (qi in {0..3}). Each core owns 512
query rows: the two 256-row chunks {qi, 7-qi} (paired so the causal workload
is identical on every core), plus the 512 contiguous sequence rows
[512*qi, 512*qi+512) for K/V projection. K/V shards are exchanged with one
AllGather per attention over the 4-core batch group, eliminating the 4x
replicated K/V projection work. The CA gather depends only on `enc`, so it
overlaps the whole SA phase.

Self-attention exploits causality with a uniform instruction schedule (the
same compiled program runs on all 8 cores; per-core variation lives in the
input data only): the first query chunk (A, rows 256*qi..) is processed
against key chunks 0..7 only, the second (B, rows 256*(7-qi)..) against all
16. Key chunks 0..7 multiply a host-provided mask on the A columns, chunks
8..15 on the B columns; everything else is exact by schedule construction.

All matmuls run in bf16 (weights converted on host, activations cast for
free inside PSUM evictions); accumulation stays fp32 in PSUM, layernorm
statistics and the softmax normalizer stay fp32. Softmax is exp(s/8 - 4)
with the normalizer from an extra ones-column on V (M=65 matmul), divided
out after accumulation.
"""

import numpy as np
import ml_dtypes

import concourse.bass as bass
import concourse.mybir as mybir
import concourse.tile as tile
from concourse import bacc
from concourse.bass import ds
from concourse.bass_utils import run_bass_kernel_spmd

F32 = mybir.dt.float32
F32R = mybir.dt.float32r
BF16 = mybir.dt.bfloat16
AF = mybir.ActivationFunctionType
ALU = mybir.AluOpType

B, S, D, H, DK, DFF = 2, 2048, 1024, 16, 64, 4096
NCORES = 8
QS = 512            # query rows per core (two 256-row chunks)
KS = 512            # kv rows projected per core
DC = D // 128       # 8 d-chunks
FC = DFF // 128     # 32 dff-chunks
PANEL = 512         # kpos panel size
NPANEL = S // PANEL # 4
NSC = PANEL // 128  # 4 kpos chunks per panel
NHP = H // 2        # 8 head pairs
KVFLAT = D * KS     # flat elems of one K^T (or V) shard
LN_EPS = 1e-5
EXP_BIAS = -4.0     # exp(s/8 - 4): overflow safety; cancels in the normalizer

NPBF = ml_dtypes.bfloat16


def _build(tap=None):
    nc = bacc.Bacc("TRN2", target_bir_lowering=False, debug=False,
                   num_devices=NCORES)

    def inp(name, shape, dt=BF16):
        return nc.dram_tensor(name, shape, dt, kind="ExternalInput").ap()

    xoT = inp("xoT", [D, QS])          # x[b].T own query cols [A|B]
    xkT = inp("xkT", [D, KS])          # x[b].T own kv rows
    ekT = inp("ekT", [D, KS])          # enc[b].T own kv rows
    msk = inp("msk", [128, 16, 256])   # causal masks: kc<8 on A cols, kc>=8 on B
    vms = inp("vms", [128, S // 128], F32)  # src_mask full, per-partition layout
    w_sa = {t: inp(f"w_sa{t}", [D, D]) for t in "qkvo"}
    w_ca = {t: inp(f"w_ca{t}", [D, D]) for t in "qkvo"}
    w_ff1 = inp("w_ff1", [D, DFF])
    w_ff2 = inp("w_ff2", [DFF, D])
    fb1 = inp("fb1", [128, FC], F32)   # ff_b1 in [128, chunk] layout
    fb2 = inp("fb2", [128, DC], F32)
    lnb = inp("lnb", [128, 6 * DC], F32)  # g1,b1,g2,b2,g3,b3 packed
    outT = nc.dram_tensor("outT", [128, DC, QS], F32, kind="ExternalOutput").ap()
    dbg = nc.dram_tensor("dbg", [128, 40, QS], F32, kind="ExternalOutput").ap() \
        if tap else None
    tapped = []

    def tapit(name, ap):
        if tap and (tap == "all" or name in tap):
            tapped.append((name, ap))

    with tile.TileContext(nc) as tc:
        with tc.tile_pool(name="glob", bufs=1) as G, \
             tc.tile_pool(name="acts", bufs=2) as ACTS, \
             tc.tile_pool(name="dram", bufs=1, space="DRAM") as DP, \
             tc.tile_pool(name="ps2", bufs=2, space="PSUM") as PS2, \
             tc.tile_pool(name="ps1", bufs=1, space="PSUM") as PS1:

            ones64 = G.tile([128, 64], BF16)
            nc.vector.memset(ones64[:], 1.0)
            onesf_t = G.tile([128, 1], F32)
            nc.vector.memset(onesf_t[:], 1.0)
            ones_f = G.tile([128, 1], F32R)
            nc.vector.tensor_copy(ones_f[:], onesf_t[:])
            cexpb = G.tile([128, 1], F32)
            nc.vector.memset(cexpb[:], EXP_BIAS)
            cleps = G.tile([128, 1], F32)
            nc.vector.memset(cleps[:], LN_EPS)
            lnbt = G.tile([128, 6 * DC], F32)
            nc.sync.dma_start(lnbt[:], lnb)
            fb1t = G.tile([128, FC], F32)
            nc.sync.dma_start(fb1t[:], fb1)
            fb2t = G.tile([128, DC], F32)
            nc.sync.dma_start(fb2t[:], fb2)
            vmst = G.tile([128, S // 128], F32)
            nc.sync.dma_start(vmst[:], vms)
            mskt = G.tile([128, 16, 256], BF16)
            nc.sync.dma_start(mskt[:], msk)
            stats = G.tile([1, 8, QS], F32)

            xo = G.tile([128, DC, QS], BF16)
            nc.sync.dma_start(xo[:], xoT.rearrange("(c p) n -> p c n", p=128))

            # K/V shard + gathered buffers (row 0 = K^T flat, row 1 = V flat)
            sa_sh = DP.tile([2, KVFLAT], BF16)
            sa_full = DP.tile([2 * 4, KVFLAT], BF16)
            ca_sh = DP.tile([2, KVFLAT], BF16)
            ca_full = DP.tile([2 * 4, KVFLAT], BF16)

            def proj_from_dram(wdram, rhs, evict, n_mc=DC, n_kc=DC, wtag="w"):
                """psum[mc] = sum_kc w[kc,mc-chunk].T @ rhs[:,kc,:]; evict(mc, psum)."""
                wre = wdram.rearrange("(c p) m -> p c m", p=128)
                for mc in range(n_mc):
                    ps = PS2.tile([128, QS], F32, tag="pj")
                    for k4 in range(n_kc // 4):
                        wt = WPOOL.tile([128, 4, 128], BF16, tag=wtag)
                        nc.sync.dma_start(
                            wt[:], wre[:, 4 * k4:4 * k4 + 4, ds(mc * 128, 128)])
                        for j in range(4):
                            kc = 4 * k4 + j
                            nc.tensor.matmul(ps[:], wt[:, j, :], rhs[:, kc, :],
                                             start=(kc == 0), stop=(kc == n_kc - 1))
                    evict(mc, ps)

            def kv_proj_store(src, wk, wv, sh):
                """Project own-row K^T and V shards (bf16) and stage to DRAM."""
                nonlocal WPOOL
                with tc.tile_pool(name="kvs", bufs=1) as KV, \
                     tc.tile_pool(name="wkv", bufs=4) as WPOOL:
                    KsT = KV.tile([128, DC, KS], BF16)

                    def evk(mc, ps):
                        nc.vector.tensor_copy(KsT[:, mc, :], ps[:])
                    proj_from_dram(wk, src, evk)
                    nc.sync.dma_start(
                        sh[:].rearrange("two (c p t) -> two p c t", c=DC, p=128)[0],
                        KsT[:])

                    Vs = KV.tile([128, NSC, D], BF16)
                    wvre = wv.rearrange("(c p) m -> p c m", p=128)
                    for nh in range(2):
                        pss = [PS2.tile([128, QS], F32, tag="s0", name="vps0"),
                               PS2.tile([128, QS], F32, tag="s1", name="vps1"),
                               PS1.tile([128, QS], F32, tag="o0", name="vps2"),
                               PS1.tile([128, QS], F32, tag="o1", name="vps3")]
                        for kc in range(DC):
                            wvt = WPOOL.tile([128, PANEL], BF16, tag="wv")
                            nc.sync.dma_start(
                                wvt[:], wvre[:, kc, ds(nh * 512, 512)])
                            for sc in range(NSC):
                                nc.tensor.matmul(
                                    pss[sc][:], src[:, kc, ds(sc * 128, 128)],
                                    wvt[:], start=(kc == 0), stop=(kc == DC - 1))
                        for sc in range(NSC):
                            nc.vector.tensor_copy(
                                Vs[:, sc, ds(nh * 512, 512)], pss[sc][:])
                    nc.sync.dma_start(
                        sh[:].rearrange("two (sc p m) -> two p sc m",
                                        sc=NSC, p=128)[1],
                        Vs[:])

            def layernorm(xpre, gcol, bcol, out, TMP):
                """out[:,mc,:] = (xpre - mu)/sd * g + b, stats over d."""
                pmu = PS2.tile([1, QS], F32, tag="pj")
                for kc in range(DC):
                    nc.tensor.matmul(pmu[:], ones_f[:], xpre[:, kc, :],
                                     start=(kc == 0), stop=(kc == DC - 1))
                pm2 = PS2.tile([1, QS], F32, tag="pj")
                for kc in range(DC):
                    sq = TMP.tile([128, QS], F32R, tag="sq")
                    nc.scalar.activation(sq[:], xpre[:, kc, :], AF.Square)
                    nc.tensor.matmul(pm2[:], ones_f[:], sq[:],
                                     start=(kc == 0), stop=(kc == DC - 1))
                mu = stats[0:1, 0, :]
                ex2 = stats[0:1, 1, :]
                var = stats[0:1, 2, :]
                sd = stats[0:1, 3, :]
                rstd = stats[0:1, 4, :]
                nc.vector.tensor_scalar_mul(mu, pmu[:], 1.0 / D)
                nc.vector.tensor_scalar_mul(ex2, pm2[:], 1.0 / D)
                nc.vector.tensor_tensor(var, mu, mu, ALU.mult)
                nc.vector.tensor_sub(var, ex2, var)
                nc.scalar.activation(sd, var, AF.Sqrt, bias=cleps[0:1, :])
                nc.vector.reciprocal(rstd, sd)
                mub = TMP.tile([128, QS], F32, tag="mub")
                nc.gpsimd.partition_broadcast(mub[:], mu)
                rsb = TMP.tile([128, QS], F32, tag="rsb")
                nc.gpsimd.partition_broadcast(rsb[:], rstd)
                for mc in range(DC):
                    t = TMP.tile([128, QS], F32, tag="t")
                    nc.vector.tensor_sub(t[:], xpre[:, mc, :], mub[:])
                    nc.vector.tensor_mul(t[:], t[:], rsb[:])
                    nc.vector.tensor_scalar(
                        out=out[:, mc, :], in0=t[:],
                        scalar1=gcol[:, mc:mc + 1], scalar2=bcol[:, mc:mc + 1],
                        op0=ALU.mult, op1=ALU.add)

            def attention(qsrc, full, w, res, gcol, bcol, masked):
                nonlocal WPOOL
                sfx = "sa" if masked else "ca"
                kfull = full[:].rearrange("(r two) (c p t) -> r two p c t",
                                          two=2, c=DC, p=128)
                vfull = full[:].rearrange("(r two) (sc p h k) -> r two p sc h k",
                                          two=2, sc=NSC, p=128, h=H)
                with tc.tile_pool(name="attn", bufs=1) as A:
                    QT = A.tile([128, DC, QS], BF16)
                    oacc = A.tile([65, H, QS], F32)
                    with tc.tile_pool(name="wq", bufs=4) as WPOOL:
                        def evq(mc, ps):
                            nc.vector.tensor_copy(QT[:, mc, :], ps[:])
                        proj_from_dram(w["q"], qsrc, evq)
                    tapit("QT" + sfx, QT)

                    with tc.tile_pool(name="panel", bufs=2) as P, \
                         tc.tile_pool(name="pp", bufs=2) as PP:
                        for p in range(NPANEL):
                            # SA causal schedule: A cols only against panels 0,1
                            a_on = (not masked) or p < 2
                            cols = ds(0, QS) if a_on else ds(256, 256)
                            KT = P.tile([128, DC, PANEL], BF16, tag="kt")
                            nc.sync.dma_start(KT[:], kfull[p, 0])
                            v1 = P.tile([128, NSC, H, DK + 1], BF16, tag="v1")
                            for sc in range(NSC):
                                nc.sync.dma_start(v1[:, sc, :, 0:DK],
                                                  vfull[p, 1, :, sc])
                            nc.vector.tensor_copy(
                                v1[:, :, :, DK],
                                ones64[:].rearrange("p (a b) -> p a b", a=NSC))
                            if not masked:
                                # fold src_mask into V rows (incl. ones column)
                                for sc in range(NSC):
                                    nc.vector.tensor_scalar_mul(
                                        v1[:, sc, :, :], v1[:, sc, :, :],
                                        vmst[:, p * NSC + sc:p * NSC + sc + 1])
                            if p == 0:
                                tapit("KT" + sfx, KT)
                            for hp in range(NHP):
                                po0 = PS1.tile([65, QS], F32, tag="o0")
                                po1 = PS1.tile([65, QS], F32, tag="o1")
                                for sc in range(NSC):
                                    kc128 = 4 * p + sc
                                    ps0 = PS2.tile([128, QS], F32, tag="s0")
                                    ps1 = PS2.tile([128, QS], F32, tag="s1")
                                    nc.tensor.matmul(
                                        ps0[:, cols], KT[0:64, hp, ds(sc * 128, 128)],
                                        QT[0:64, hp, cols], start=True, stop=True)
                                    nc.tensor.matmul(
                                        ps1[:, cols], KT[64:128, hp, ds(sc * 128, 128)],
                                        QT[64:128, hp, cols], start=True, stop=True,
                                        tile_position=(64, 0))
                                    p0 = PP.tile([128, QS], BF16, tag="p0")
                                    p1 = PP.tile([128, QS], BF16, tag="p1")
                                    nc.scalar.activation(p0[:, cols], ps0[:, cols],
                                                         AF.Exp,
                                                         scale=0.125, bias=cexpb[:])
                                    nc.scalar.activation(p1[:, cols], ps1[:, cols],
                                                         AF.Exp,
                                                         scale=0.125, bias=cexpb[:])
                                    if masked:
                                        # kc<8: causal boundary in A cols;
                                        # kc>=8: in B cols (A cols not computed)
                                        mcols = ds(0, 256) if kc128 < 8 \
                                            else ds(256, 256)
                                        nc.vector.tensor_mul(
                                            p0[:, mcols], p0[:, mcols],
                                            mskt[:, kc128, :])
                                        nc.vector.tensor_mul(
                                            p1[:, mcols], p1[:, mcols],
                                            mskt[:, kc128, :])
                                    nc.tensor.matmul(po0[:, cols],
                                                     v1[:, sc, 2 * hp, :],
                                                     p0[:, cols], start=(sc == 0),
                                                     stop=(sc == NSC - 1))
                                    nc.tensor.matmul(po1[:, cols],
                                                     v1[:, sc, 2 * hp + 1, :],
                                                     p1[:, cols], start=(sc == 0),
                                                     stop=(sc == NSC - 1))
                                if p == 0:
                                    nc.vector.tensor_copy(oacc[:, 2 * hp, :], po0[:])
                                    nc.vector.tensor_copy(oacc[:, 2 * hp + 1, :],
                                                          po1[:])
                                else:
                                    nc.vector.tensor_add(
                                        oacc[:, 2 * hp, cols],
                                        oacc[:, 2 * hp, cols], po0[:, cols])
                                    nc.vector.tensor_add(
                                        oacc[:, 2 * hp + 1, cols],
                                        oacc[:, 2 * hp + 1, cols], po1[:, cols])

                    with tc.tile_pool(name="aepi", bufs=1) as E, \
                         tc.tile_pool(name="rnbp", bufs=2) as RNB, \
                         tc.tile_pool(name="tmp", bufs=2) as TMP, \
                         tc.tile_pool(name="wo", bufs=4) as WPOOL:
                        tapit("oacc" + sfx, oacc)
                        rn = E.tile([1, H, QS], F32)
                        nc.vector.reciprocal(rn[:], oacc[64:65, :, :])
                        ON = E.tile([128, DC, QS], BF16)
                        for m in range(DC):
                            rnb = RNB.tile([64, 2, QS], F32, tag="rnb")
                            nc.gpsimd.partition_broadcast(rnb[:, 0, :],
                                                          rn[0:1, 2 * m, :])
                            nc.gpsimd.partition_broadcast(rnb[:, 1, :],
                                                          rn[0:1, 2 * m + 1, :])
                            nc.vector.tensor_mul(ON[0:64, m, :],
                                                 oacc[0:64, 2 * m, :], rnb[:, 0, :])
                            nc.vector.tensor_mul(ON[64:128, m, :],
                                                 oacc[0:64, 2 * m + 1, :],
                                                 rnb[:, 1, :])
                        xpre = E.tile([128, DC, QS], F32R)

                        def evo(mc, ps):
                            nc.vector.tensor_add(xpre[:, mc, :], ps[:], res[:, mc, :])
                        proj_from_dram(w["o"], ON, evo)
                        tapit("ON" + sfx, ON)
                        tapit("xpre" + sfx, xpre)
                        xnext = ACTS.tile([128, DC, QS], BF16, tag="act")
                        layernorm(xpre, gcol, bcol, xnext, TMP)
                        tapit("xn" + sfx, xnext)
                return xnext

            WPOOL = None
            g1, b1 = lnbt[:, 0:DC], lnbt[:, DC:2 * DC]
            g2, b2 = lnbt[:, 2 * DC:3 * DC], lnbt[:, 3 * DC:4 * DC]
            g3, b3 = lnbt[:, 4 * DC:5 * DC], lnbt[:, 5 * DC:6 * DC]

            # ---- K/V shards + gathers (CA gather overlaps all of SA) ----
            with tc.tile_pool(name="kvsrc", bufs=1) as KSRC:
                xk = KSRC.tile([128, DC, KS], BF16)
                nc.sync.dma_start(xk[:], xkT.rearrange("(c p) n -> p c n", p=128))
                kv_proj_store(xk, w_sa["k"], w_sa["v"], sa_sh)
                nc.gpsimd.collective_compute(
                    "AllGather", ALU.bypass,
                    replica_groups=[[0, 1, 2, 3], [4, 5, 6, 7]],
                    ins=[sa_sh[:].opt()], outs=[sa_full[:].opt()])
                ek = KSRC.tile([128, DC, KS], BF16)
                nc.sync.dma_start(ek[:], ekT.rearrange("(c p) n -> p c n", p=128))
                kv_proj_store(ek, w_ca["k"], w_ca["v"], ca_sh)
                nc.gpsimd.collective_compute(
                    "AllGather", ALU.bypass,
                    replica_groups=[[0, 1, 2, 3], [4, 5, 6, 7]],
                    ins=[ca_sh[:].opt()], outs=[ca_full[:].opt()])

            x1 = attention(xo, sa_full, w_sa, xo, g1, b1, masked=True)
            x2 = attention(x1, ca_full, w_ca, x1, g2, b2, masked=False)

            # ---- FFN ----
            with tc.tile_pool(name="ffn", bufs=1) as F, \
                 tc.tile_pool(name="tmp2", bufs=2) as TMP, \
                 tc.tile_pool(name="wf", bufs=4) as WPOOL:
                h1 = F.tile([128, FC, QS], BF16)

                def ev1(fc, ps):
                    nc.scalar.activation(h1[:, fc, :], ps[:], AF.Relu,
                                         bias=fb1t[:, fc:fc + 1])
                proj_from_dram(w_ff1, x2, ev1, n_mc=FC, n_kc=DC)

                tapit("h1a", h1[:, 0:8, :])
                xpre = F.tile([128, DC, QS], F32R)

                def ev2(mc, ps):
                    nc.vector.scalar_tensor_tensor(
                        out=xpre[:, mc, :], in0=ps[:],
                        scalar=fb2t[:, mc:mc + 1], in1=x2[:, mc, :],
                        op0=ALU.add, op1=ALU.add)
                proj_from_dram(w_ff2, h1, ev2, n_mc=DC, n_kc=FC)

                tapit("xpreff", xpre)
                out = F.tile([128, DC, QS], F32)
                layernorm(xpre, g3, b3, out, TMP)
                tapit("outf", out)
                tc.strict_bb_all_engine_barrier()
                for mc in range(DC):
                    nc.sync.dma_start(outT[:, mc, :], out[:, mc, :])
            if tap:
                base = 0
                tap_layout.clear()
                for name, t in tapped:
                    sh = t.shape
                    nparts = sh[0]
                    assert len(sh) == 3 and sh[2] == QS
                    tap_layout[name] = (base, sh[1], nparts)
                    for cci in range(sh[1]):
                        nc.sync.dma_start(
                            dbg[0:nparts, base + cci, :].bitcast(t.dtype),
                            t[:, cci, :])
                    base += sh[1]
                assert base <= 40

    nc.compile()
    return nc


tap_layout = {}
_NC_CACHE = None


def _get_nc():
    global _NC_CACHE
    if _NC_CACHE is None:
        _NC_CACHE = _build()
    return _NC_CACHE


def _prep_in_maps(x, enc, tgt_mask, src_mask,
                  sa_wq, sa_wk, sa_wv, sa_wo,
                  ca_wq, ca_wk, ca_wv, ca_wo,
                  ff_w1, ff_b1, ff_w2, ff_b2,
                  ln1_g, ln1_b, ln2_g, ln2_b, ln3_g, ln3_b):
    f32 = np.float32

    def c(a):
        return np.ascontiguousarray(np.asarray(a), dtype=f32)

    def cb(a):
        return np.ascontiguousarray(np.asarray(a, dtype=f32).astype(NPBF))

    xTb = [np.asarray(x, dtype=f32)[b].T for b in range(B)]   # [1024, 2048]
    eTb = [np.asarray(enc, dtype=f32)[b].T for b in range(B)]
    tm = np.asarray(tgt_mask)[0, 0].astype(f32).T             # [k, q]
    sm = np.asarray(src_mask)[0, 0, 0].astype(f32)            # [k]
    vms = c(sm.reshape(S // 128, 128).T)                      # [128, 16]

    def percol(v, nchunks):
        return c(np.asarray(v).reshape(nchunks, 128).T)

    lnb = c(np.concatenate(
        [percol(v, DC) for v in [ln1_g, ln1_b, ln2_g, ln2_b, ln3_g, ln3_b]],
        axis=1))
    fb1 = percol(ff_b1, FC)
    fb2 = percol(ff_b2, DC)
    shared = {
        "vms": vms, "lnb": lnb, "fb1": fb1, "fb2": fb2,
        "w_saq": cb(sa_wq), "w_sak": cb(sa_wk), "w_sav": cb(sa_wv),
        "w_sao": cb(sa_wo),
        "w_caq": cb(ca_wq), "w_cak": cb(ca_wk), "w_cav": cb(ca_wv),
        "w_cao": cb(ca_wo),
        "w_ff1": cb(ff_w1), "w_ff2": cb(ff_w2),
    }
    in_maps = []
    for core in range(NCORES):
        b, qi = core // 4, core % 4
        cA, cB = qi, 7 - qi
        qcols = np.r_[256 * cA:256 * cA + 256, 256 * cB:256 * cB + 256]
        m = dict(shared)
        m["xoT"] = cb(xTb[b][:, qcols])
        m["xkT"] = cb(xTb[b][:, 512 * qi:512 * qi + 512])
        m["ekT"] = cb(eTb[b][:, 512 * qi:512 * qi + 512])
        # masks: tiles 0..7 = A cols vs key chunks 0..7;
        #        tiles 8..15 = B cols vs key chunks 8..15
        mk = np.empty((128, 16, 256), f32)
        for kc in range(8):
            mk[:, kc, :] = tm[128 * kc:128 * kc + 128,
                              256 * cA:256 * cA + 256]
        for kc in range(8, 16):
            mk[:, kc, :] = tm[128 * kc:128 * kc + 128,
                              256 * cB:256 * cB + 256]
        m["msk"] = np.ascontiguousarray(mk.astype(NPBF))
        in_maps.append(m)
    return in_maps


def _gather_out(res):
    out = np.empty((B, S, D), dtype=np.float32)
    for core in range(NCORES):
        b, qi = core // 4, core % 4
        cA, cB = qi, 7 - qi
        arr = res.results[core]["outT"]  # [128, DC, QS]
        full = arr.transpose(1, 0, 2).reshape(D, QS).T  # [512, 1024]
        out[b, 256 * cA:256 * cA + 256, :] = full[0:256]
        out[b, 256 * cB:256 * cB + 256, :] = full[256:512]
    return out


def kernel(**inputs):
    in_maps = _prep_in_maps(**inputs)
    nc = _get_nc()
    res = run_bass_kernel_spmd(nc, in_maps, core_ids=list(range(NCORES)))
    return _gather_out(res)


def _profiled_run(inputs):
    """Test-only: run with NTFF tracing to get HW exec time."""
    in_maps = _prep_in_maps(**inputs)
    nc = _get_nc()
    return run_bass_kernel_spmd(nc, in_maps, core_ids=list(range(NCORES)),
                                trace=True)


# revision 10
# speedup vs baseline: 1.3114x; 1.0626x over previous
"""Trainium2 Bass kernel for nn_DecoderBlock (self-attn + cross-attn + FFN, post-LN).

Sharding: 8 cores = (batch b in {0,1}) x (qi in {0..3}). Each core owns 512
query rows: the two 256-row chunks {qi, 7-qi} (paired so the causal workload
is identical on every core), plus the 512 contiguous sequence rows
[512*qi, 512*qi+512) for K/V projection. K/V shards are exchanged with one
AllGather per attention over the 4-core batch group, eliminating the 4x
replicated K/V projection work. The CA gather depends only on `enc`, so it
overlaps the whole SA phase.

Self-attention exploits causality with a uniform instruction schedule (the
same compiled program runs on all 8 cores; per-core variation lives in the
input data only): the first query chunk (A, rows 256*qi..) is processed
against key chunks 0..7 only, the second (B, rows 256*(7-qi)..) against all
16. Key chunks 0..7 multiply a host-provided mask on the A columns, chunks
8..15 on the B columns; everything else is exact by schedule construction.

All matmuls run in bf16 (weights converted on host, activations cast for
free inside PSUM evictions); accumulation stays fp32 in PSUM, layernorm
statistics and the softmax normalizer stay fp32. Softmax is exp(s/8 - 4)
with the normalizer from an extra ones-column on V (M=65 matmul), divided
out after accumulation.
"""

import numpy as np
import ml_dtypes

import concourse.bass as bass
import concourse.mybir as mybir
import concourse.tile as tile
from concourse import bacc
from concourse.bass import ds
from concourse.bass_utils import run_bass_kernel_spmd

F32 = mybir.dt.float32
F32R = mybir.dt.float32r
BF16 = mybir.dt.bfloat16
AF = mybir.ActivationFunctionType
ALU = mybir.AluOpType

B, S, D, H, DK, DFF = 2, 2048, 1024, 16, 64, 4096
NCORES = 8
QS = 512            # query rows per core (two 256-row chunks)
KS = 512            # kv rows projected per core
DC = D // 128       # 8 d-chunks
FC = DFF // 128     # 32 dff-chunks
PANEL = 512         # kpos panel size
NPANEL = S // PANEL # 4
NSC = PANEL // 128  # 4 kpos chunks per panel
NHP = H // 2        # 8 head pairs
KVFLAT = D * KS     # flat elems of one K^T (or V) shard
LN_EPS = 1e-5
EXP_BIAS = -4.0     # exp(s/8 - 4): overflow safety; cancels in the normalizer

NPBF = ml_dtypes.bfloat16


def _build(tap=None):
    nc = bacc.Bacc("TRN2", target_bir_lowering=False, debug=False,
                   num_devices=NCORES)

    def inp(name, shape, dt=BF16):
        return nc.dram_tensor(name, shape, dt, kind="ExternalInput").ap()

    xoT = inp("xoT", [D, QS])          # x[b].T own query cols [A|B]
    xkT = inp("xkT", [D, KS])          # x[b].T own kv rows
    ekT = inp("ekT", [D, KS])          # enc[b].T own kv rows
    msk = inp("msk", [128, 16, 256])   # causal masks: kc<8 on A cols, kc>=8 on B
    vms = inp("vms", [128, S // 128], F32)  # src_mask full, per-partition layout
    w_sa = {t: inp(f"w_sa{t}", [D, D]) for t in "qkvo"}
    w_ca = {t: inp(f"w_ca{t}", [D, D]) for t in "qkvo"}
    w_ff1 = inp("w_ff1", [D, DFF])
    w_ff2 = inp("w_ff2", [DFF, D])
    fb1 = inp("fb1", [128, FC], F32)   # ff_b1 in [128, chunk] layout
    fb2 = inp("fb2", [128, DC], F32)
    lnb = inp("lnb", [128, 6 * DC], F32)  # g1,b1,g2,b2,g3,b3 packed
    outT = nc.dram_tensor("outT", [128, DC, QS], F32, kind="ExternalOutput").ap()
    dbg = nc.dram_tensor("dbg", [128, 40, QS], F32, kind="ExternalOutput").ap() \
        if tap else None
    tapped = []

    def tapit(name, ap):
        if tap and (tap == "all" or name in tap):
            tapped.append((name, ap))

    with tile.TileContext(nc) as tc:
        with tc.tile_pool(name="glob", bufs=1) as G, \
             tc.tile_pool(name="acts", bufs=2) as ACTS, \
             tc.tile_pool(name="dram", bufs=1, space="DRAM") as DP, \
             tc.tile_pool(name="ps2", bufs=2, space="PSUM") as PS2, \
             tc.tile_pool(name="ps1", bufs=1, space="PSUM") as PS1:

            ones64 = G.tile([128, 64], BF16)
            nc.vector.memset(ones64[:], 1.0)
            onesf_t = G.tile([128, 1], F32)
            nc.vector.memset(onesf_t[:], 1.0)
            ones_f = G.tile([128, 1], F32R)
            nc.vector.tensor_copy(ones_f[:], onesf_t[:])
            onesrow_t = G.tile([1, 128], F32)
            nc.vector.memset(onesrow_t[:], 1.0)
            onesrow = G.tile([1, 128], F32R)
            nc.vector.tensor_copy(onesrow[:], onesrow_t[:])
            cexpb = G.tile([128, 1], F32)
            nc.vector.memset(cexpb[:], EXP_BIAS)
            cleps = G.tile([128, 1], F32)
            nc.vector.memset(cleps[:], LN_EPS)
            lnbt = G.tile([128, 6 * DC], F32)
            nc.sync.dma_start(lnbt[:], lnb)
            fb1t = G.tile([128, FC], F32)
            nc.sync.dma_start(fb1t[:], fb1)
            fb2t = G.tile([128, DC], F32)
            nc.sync.dma_start(fb2t[:], fb2)
            vmst = G.tile([128, S // 128], F32)
            nc.sync.dma_start(vmst[:], vms)
            mskt = G.tile([128, 16, 256], BF16)
            nc.sync.dma_start(mskt[:], msk)
            stats = G.tile([1, 8, QS], F32R)

            xo = G.tile([128, DC, QS], BF16)
            nc.sync.dma_start(xo[:], xoT.rearrange("(c p) n -> p c n", p=128))

            # K/V shard + gathered buffers (row 0 = K^T flat, row 1 = V flat)
            sa_ksh = DP.tile([1, KVFLAT], BF16)
            sa_kfull = DP.tile([4, KVFLAT], BF16)
            sa_vsh = DP.tile([1, KVFLAT], BF16)
            sa_vfull = DP.tile([4, KVFLAT], BF16)
            ca_ksh = DP.tile([1, KVFLAT], BF16)
            ca_kfull = DP.tile([4, KVFLAT], BF16)
            ca_vsh = DP.tile([1, KVFLAT], BF16)
            ca_vfull = DP.tile([4, KVFLAT], BF16)

            RG = [[0, 1, 2, 3], [4, 5, 6, 7]]

            def gather(sh, full):
                nc.gpsimd.collective_compute(
                    "AllGather", ALU.bypass, replica_groups=RG,
                    ins=[sh[:].opt()], outs=[full[:].opt()])

            def proj_from_dram(wdram, rhs, evict, n_mc=DC, n_kc=DC, wtag="w"):
                """psum[mc] = sum_kc w[kc,mc-chunk].T @ rhs[:,kc,:]; evict(mc, psum)."""
                wre = wdram.rearrange("(c p) m -> p c m", p=128)
                for mc in range(n_mc):
                    ps = PS2.tile([128, QS], F32, tag="pj")
                    for k4 in range(n_kc // 4):
                        wt = WPOOL.tile([128, 4, 128], BF16, tag=wtag)
                        nc.sync.dma_start(
                            wt[:], wre[:, 4 * k4:4 * k4 + 4, ds(mc * 128, 128)])
                        for j in range(4):
                            kc = 4 * k4 + j
                            nc.tensor.matmul(ps[:], wt[:, j, :], rhs[:, kc, :],
                                             start=(kc == 0), stop=(kc == n_kc - 1))
                    evict(mc, ps)

            def kv_proj_store(src, wk, wv, ksh, kfull, vsh):
                """Project own-row K^T and V shards (bf16) and stage to DRAM."""
                nonlocal WPOOL
                with tc.tile_pool(name="kvs", bufs=1) as KV, \
                     tc.tile_pool(name="wkv", bufs=4) as WPOOL:
                    KsT = KV.tile([128, DC, KS], BF16)

                    def evk(mc, ps):
                        nc.vector.tensor_copy(KsT[:, mc, :], ps[:])
                    proj_from_dram(wk, src, evk)
                    nc.sync.dma_start(
                        ksh[:].rearrange("one (c p t) -> one p c t", c=DC, p=128)[0],
                        KsT[:])
                    gather(ksh, kfull)

                    Vs = KV.tile([128, NSC, D], BF16)
                    wvre = wv.rearrange("(c p) m -> p c m", p=128)
                    for nh in range(2):
                        pss = [PS2.tile([128, QS], F32, tag="s0", name="vps0"),
                               PS2.tile([128, QS], F32, tag="s1", name="vps1"),
                               PS1.tile([128, QS], F32, tag="o0", name="vps2"),
                               PS1.tile([128, QS], F32, tag="o1", name="vps3")]
                        for kc in range(DC):
                            wvt = WPOOL.tile([128, PANEL], BF16, tag="wv")
                            nc.sync.dma_start(
                                wvt[:], wvre[:, kc, ds(nh * 512, 512)])
                            for sc in range(NSC):
                                nc.tensor.matmul(
                                    pss[sc][:], src[:, kc, ds(sc * 128, 128)],
                                    wvt[:], start=(kc == 0), stop=(kc == DC - 1))
                        for sc in range(NSC):
                            nc.vector.tensor_copy(
                                Vs[:, sc, ds(nh * 512, 512)], pss[sc][:])
                    nc.sync.dma_start(
                        vsh[:].rearrange("one (sc p m) -> one p sc m",
                                         sc=NSC, p=128)[0],
                        Vs[:])

            def layernorm(xpre, gcol, bcol, out, TMP):
                """out[:,mc,:] = (xpre - mu)/sd * g + b, stats over d."""
                pmu = PS2.tile([1, QS], F32, tag="pj")
                for kc in range(DC):
                    nc.tensor.matmul(pmu[:], ones_f[:], xpre[:, kc, :],
                                     start=(kc == 0), stop=(kc == DC - 1))
                pm2 = PS2.tile([1, QS], F32, tag="pj")
                for kc in range(DC):
                    sq = TMP.tile([128, QS], F32R, tag="sq")
                    nc.scalar.activation(sq[:], xpre[:, kc, :], AF.Square)
                    nc.tensor.matmul(pm2[:], ones_f[:], sq[:],
                                     start=(kc == 0), stop=(kc == DC - 1))
                mu = stats[0:1, 0, :]
                ex2 = stats[0:1, 1, :]
                var = stats[0:1, 2, :]
                sd = stats[0:1, 3, :]
                rstd = stats[0:1, 4, :]
                with nc.allow_low_precision(reason="f32r is f32 bits"):
                    nc.vector.tensor_scalar_mul(mu, pmu[:], 1.0 / D)
                    nc.vector.tensor_scalar_mul(ex2, pm2[:], 1.0 / D)
                    nc.vector.tensor_tensor(var, mu, mu, ALU.mult)
                    nc.vector.tensor_sub(var, ex2, var)
                    nc.scalar.activation(sd, var, AF.Sqrt, bias=cleps[0:1, :])
                    nc.vector.reciprocal(rstd, sd)
                mub = PS1.tile([128, QS], F32, tag="o0")
                nc.tensor.matmul(mub[:], onesrow[:], mu, start=True, stop=True)
                rsb = PS1.tile([128, QS], F32, tag="o1")
                nc.tensor.matmul(rsb[:], onesrow[:], rstd, start=True, stop=True)
                for mc in range(DC):
                    t = TMP.tile([128, QS], F32, tag="t")
                    nc.vector.tensor_sub(t[:], xpre[:, mc, :], mub[:])
                    nc.vector.tensor_mul(t[:], t[:], rsb[:])
                    nc.vector.tensor_scalar(
                        out=out[:, mc, :], in0=t[:],
                        scalar1=gcol[:, mc:mc + 1], scalar2=bcol[:, mc:mc + 1],
                        op0=ALU.mult, op1=ALU.add)

            def attention(qsrc, full, w, res, gcol, bcol, masked):
                nonlocal WPOOL
                sfx = "sa" if masked else "ca"
                kfull = full[0][:].rearrange("r (c p t) -> r p c t",
                                             c=DC, p=128)
                vfull = full[1][:].rearrange("r (sc p h k) -> r p sc h k",
                                             sc=NSC, p=128, h=H)
                with tc.tile_pool(name="attn", bufs=1) as A:
                    QT = A.tile([128, DC, QS], BF16)
                    oacc = A.tile([65, H, QS], F32)
                    with tc.tile_pool(name="wq", bufs=4) as WPOOL:
                        def evq(mc, ps):
                            nc.vector.tensor_copy(QT[:, mc, :], ps[:])
                        proj_from_dram(w["q"], qsrc, evq)
                    tapit("QT" + sfx, QT)

                    with tc.tile_pool(name="panel", bufs=2) as P, \
                         tc.tile_pool(name="pp", bufs=2) as PP:
                        for p in range(NPANEL):
                            # SA causal schedule: A cols only against panels 0,1
                            a_on = (not masked) or p < 2
                            cols = ds(0, QS) if a_on else ds(256, 256)
                            KT = P.tile([128, DC, PANEL], BF16, tag="kt")
                            nc.sync.dma_start(KT[:], kfull[p])
                            v1 = P.tile([128, NSC, H, DK + 1], BF16, tag="v1")
                            for sc in range(NSC):
                                nc.sync.dma_start(v1[:, sc, :, 0:DK],
                                                  vfull[p, :, sc])
                            nc.vector.tensor_copy(
                                v1[:, :, :, DK],
                                ones64[:].rearrange("p (a b) -> p a b", a=NSC))
                            if not masked:
                                # fold src_mask into V rows (incl. ones column)
                                for sc in range(NSC):
                                    nc.vector.tensor_scalar_mul(
                                        v1[:, sc, :, :], v1[:, sc, :, :],
                                        vmst[:, p * NSC + sc:p * NSC + sc + 1])
                            if p == 0:
                                tapit("KT" + sfx, KT)
                            for hp in range(NHP):
                                po0 = PS1.tile([65, QS], F32, tag="o0")
                                po1 = PS1.tile([65, QS], F32, tag="o1")
                                for sc in range(NSC):
                                    kc128 = 4 * p + sc
                                    ps0 = PS2.tile([128, QS], F32, tag="s0")
                                    ps1 = PS2.tile([128, QS], F32, tag="s1")
                                    nc.tensor.matmul(
                                        ps0[:, cols], KT[0:64, hp, ds(sc * 128, 128)],
                                        QT[0:64, hp, cols], start=True, stop=True)
                                    nc.tensor.matmul(
                                        ps1[:, cols], KT[64:128, hp, ds(sc * 128, 128)],
                                        QT[64:128, hp, cols], start=True, stop=True,
                                        tile_position=(64, 0))
                                    p0 = PP.tile([128, QS], BF16, tag="p0")
                                    p1 = PP.tile([128, QS], BF16, tag="p1")
                                    nc.scalar.activation(p0[:, cols], ps0[:, cols],
                                                         AF.Exp,
                                                         scale=0.125, bias=cexpb[:])
                                    nc.scalar.activation(p1[:, cols], ps1[:, cols],
                                                         AF.Exp,
                                                         scale=0.125, bias=cexpb[:])
                                    if masked:
                                        # kc<8: causal boundary in A cols;
                                        # kc>=8: in B cols (A cols not computed)
                                        mcols = ds(0, 256) if kc128 < 8 \
                                            else ds(256, 256)
                                        nc.vector.tensor_mul(
                                            p0[:, mcols], p0[:, mcols],
                                            mskt[:, kc128, :])
                                        nc.vector.tensor_mul(
                                            p1[:, mcols], p1[:, mcols],
                                            mskt[:, kc128, :])
                                    nc.tensor.matmul(po0[:, cols],
                                                     v1[:, sc, 2 * hp, :],
                                                     p0[:, cols], start=(sc == 0),
                                                     stop=(sc == NSC - 1))
                                    nc.tensor.matmul(po1[:, cols],
                                                     v1[:, sc, 2 * hp + 1, :],
                                                     p1[:, cols], start=(sc == 0),
                                                     stop=(sc == NSC - 1))
                                if p == 0:
                                    nc.vector.tensor_copy(oacc[:, 2 * hp, :], po0[:])
                                    nc.vector.tensor_copy(oacc[:, 2 * hp + 1, :],
                                                          po1[:])
                                else:
                                    nc.vector.tensor_add(
                                        oacc[:, 2 * hp, cols],
                                        oacc[:, 2 * hp, cols], po0[:, cols])
                                    nc.vector.tensor_add(
                                        oacc[:, 2 * hp + 1, cols],
                                        oacc[:, 2 * hp + 1, cols], po1[:, cols])

                    with tc.tile_pool(name="aepi", bufs=1) as E, \
                         tc.tile_pool(name="tmp", bufs=2) as TMP, \
                         tc.tile_pool(name="wo", bufs=4) as WPOOL:
                        tapit("oacc" + sfx, oacc)
                        rn = E.tile([1, H, QS], F32R)
                        with nc.allow_low_precision(reason="f32r is f32 bits"):
                            nc.vector.reciprocal(rn[:], oacc[64:65, :, :])
                        ON = E.tile([128, DC, QS], BF16)
                        for m in range(DC):
                            rb0 = PS2.tile([128, QS], F32, tag="s0")
                            rb1 = PS2.tile([128, QS], F32, tag="s1")
                            nc.tensor.matmul(rb0[0:64, :], onesrow[:, 0:64],
                                             rn[0:1, 2 * m, :],
                                             start=True, stop=True)
                            nc.tensor.matmul(rb1[0:64, :], onesrow[:, 0:64],
                                             rn[0:1, 2 * m + 1, :],
                                             start=True, stop=True)
                            nc.vector.tensor_mul(ON[0:64, m, :],
                                                 oacc[0:64, 2 * m, :], rb0[0:64, :])
                            nc.vector.tensor_mul(ON[64:128, m, :],
                                                 oacc[0:64, 2 * m + 1, :],
                                                 rb1[0:64, :])
                        xpre = E.tile([128, DC, QS], F32R)

                        def evo(mc, ps):
                            nc.vector.tensor_add(xpre[:, mc, :], ps[:], res[:, mc, :])
                        proj_from_dram(w["o"], ON, evo)
                        tapit("ON" + sfx, ON)
                        tapit("xpre" + sfx, xpre)
                        xnext = ACTS.tile([128, DC, QS], BF16, tag="act")
                        layernorm(xpre, gcol, bcol, xnext, TMP)
                        tapit("xn" + sfx, xnext)
                return xnext

            WPOOL = None
            g1, b1 = lnbt[:, 0:DC], lnbt[:, DC:2 * DC]
            g2, b2 = lnbt[:, 2 * DC:3 * DC], lnbt[:, 3 * DC:4 * DC]
            g3, b3 = lnbt[:, 4 * DC:5 * DC], lnbt[:, 5 * DC:6 * DC]

            # ---- K/V shards + gathers (CA gather overlaps all of SA) ----
            with tc.tile_pool(name="kvsrc", bufs=1) as KSRC:
                xk = KSRC.tile([128, DC, KS], BF16)
                nc.sync.dma_start(xk[:], xkT.rearrange("(c p) n -> p c n", p=128))
                kv_proj_store(xk, w_sa["k"], w_sa["v"], sa_ksh, sa_kfull, sa_vsh)
                gather(sa_vsh, sa_vfull)
                ek = KSRC.tile([128, DC, KS], BF16)
                nc.sync.dma_start(ek[:], ekT.rearrange("(c p) n -> p c n", p=128))
                kv_proj_store(ek, w_ca["k"], w_ca["v"], ca_ksh, ca_kfull, ca_vsh)
                gather(ca_vsh, ca_vfull)

            x1 = attention(xo, (sa_kfull, sa_vfull), w_sa, xo, g1, b1,
               masked=True)
            x2 = attention(x1, (ca_kfull, ca_vfull), w_ca, x1, g2, b2,
               masked=False)

            # ---- FFN ----
            with tc.tile_pool(name="ffn", bufs=1) as F, \
                 tc.tile_pool(name="tmp2", bufs=2) as TMP, \
                 tc.tile_pool(name="wf", bufs=4) as WPOOL:
                h1 = F.tile([128, FC, QS], BF16)

                def ev1(fc, ps):
                    nc.scalar.activation(h1[:, fc, :], ps[:], AF.Relu,
                                         bias=fb1t[:, fc:fc + 1])
                proj_from_dram(w_ff1, x2, ev1, n_mc=FC, n_kc=DC)

                tapit("h1a", h1[:, 0:8, :])
                xpre = F.tile([128, DC, QS], F32R)

                def ev2(mc, ps):
                    nc.vector.scalar_tensor_tensor(
                        out=xpre[:, mc, :], in0=ps[:],
                        scalar=fb2t[:, mc:mc + 1], in1=x2[:, mc, :],
                        op0=ALU.add, op1=ALU.add)
                proj_from_dram(w_ff2, h1, ev2, n_mc=DC, n_kc=FC)

                tapit("xpreff", xpre)
                out = F.tile([128, DC, QS], F32)
                layernorm(xpre, g3, b3, out, TMP)
                tapit("outf", out)
                for mc in range(DC):
                    nc.sync.dma_start(outT[:, mc, :], out[:, mc, :])
            if tap:
                base = 0
                tap_layout.clear()
                for name, t in tapped:
                    sh = t.shape
                    nparts = sh[0]
                    assert len(sh) == 3 and sh[2] == QS
                    tap_layout[name] = (base, sh[1], nparts)
                    for cci in range(sh[1]):
                        nc.sync.dma_start(
                            dbg[0:nparts, base + cci, :].bitcast(t.dtype),
                            t[:, cci, :])
                    base += sh[1]
                assert base <= 40

    nc.compile()
    return nc


tap_layout = {}
_NC_CACHE = None


def _get_nc():
    global _NC_CACHE
    if _NC_CACHE is None:
        _NC_CACHE = _build()
    return _NC_CACHE


def _prep_in_maps(x, enc, tgt_mask, src_mask,
                  sa_wq, sa_wk, sa_wv, sa_wo,
                  ca_wq, ca_wk, ca_wv, ca_wo,
                  ff_w1, ff_b1, ff_w2, ff_b2,
                  ln1_g, ln1_b, ln2_g, ln2_b, ln3_g, ln3_b):
    f32 = np.float32

    def c(a):
        return np.ascontiguousarray(np.asarray(a), dtype=f32)

    def cb(a):
        return np.ascontiguousarray(np.asarray(a, dtype=f32).astype(NPBF))

    xTb = [np.asarray(x, dtype=f32)[b].T for b in range(B)]   # [1024, 2048]
    eTb = [np.asarray(enc, dtype=f32)[b].T for b in range(B)]
    tm = np.asarray(tgt_mask)[0, 0].astype(f32).T             # [k, q]
    sm = np.asarray(src_mask)[0, 0, 0].astype(f32)            # [k]
    vms = c(sm.reshape(S // 128, 128).T)                      # [128, 16]

    def percol(v, nchunks):
        return c(np.asarray(v).reshape(nchunks, 128).T)

    lnb = c(np.concatenate(
        [percol(v, DC) for v in [ln1_g, ln1_b, ln2_g, ln2_b, ln3_g, ln3_b]],
        axis=1))
    fb1 = percol(ff_b1, FC)
    fb2 = percol(ff_b2, DC)
    shared = {
        "vms": vms, "lnb": lnb, "fb1": fb1, "fb2": fb2,
        "w_saq": cb(sa_wq), "w_sak": cb(sa_wk), "w_sav": cb(sa_wv),
        "w_sao": cb(sa_wo),
        "w_caq": cb(ca_wq), "w_cak": cb(ca_wk), "w_cav": cb(ca_wv),
        "w_cao": cb(ca_wo),
        "w_ff1": cb(ff_w1), "w_ff2": cb(ff_w2),
    }
    in_maps = []
    for core in range(NCORES):
        b, qi = core // 4, core % 4
        cA, cB = qi, 7 - qi
        qcols = np.r_[256 * cA:256 * cA + 256, 256 * cB:256 * cB + 256]
        m = dict(shared)
        m["xoT"] = cb(xTb[b][:, qcols])
        m["xkT"] = cb(xTb[b][:, 512 * qi:512 * qi + 512])
        m["ekT"] = cb(eTb[b][:, 512 * qi:512 * qi + 512])
        # masks: tiles 0..7 = A cols vs key chunks 0..7;
        #        tiles 8..15 = B cols vs key chunks 8..15
        mk = np.empty((128, 16, 256), f32)
        for kc in range(8):
            mk[:, kc, :] = tm[128 * kc:128 * kc + 128,
                              256 * cA:256 * cA + 256]
        for kc in range(8, 16):
            mk[:, kc, :] = tm[128 * kc:128 * kc + 128,
                              256 * cB:256 * cB + 256]
        m["msk"] = np.ascontiguousarray(mk.astype(NPBF))
        in_maps.append(m)
    return in_maps


def _gather_out(res):
    out = np.empty((B, S, D), dtype=np.float32)
    for core in range(NCORES):
        b, qi = core // 4, core % 4
        cA, cB = qi, 7 - qi
        arr = res.results[core]["outT"]  # [128, DC, QS]
        full = arr.transpose(1, 0, 2).reshape(D, QS).T  # [512, 1024]
        out[b, 256 * cA:256 * cA + 256, :] = full[0:256]
        out[b, 256 * cB:256 * cB + 256, :] = full[256:512]
    return out


def kernel(**inputs):
    in_maps = _prep_in_maps(**inputs)
    nc = _get_nc()
    res = run_bass_kernel_spmd(nc, in_maps, core_ids=list(range(NCORES)))
    return _gather_out(res)


def _profiled_run(inputs):
    """Test-only: run with NTFF tracing to get HW exec time."""
    in_maps = _prep_in_maps(**inputs)
    nc = _get_nc()
    return run_bass_kernel_spmd(nc, in_maps, core_ids=list(range(NCORES)),
                                trace=True)
